# revision 18
# baseline (speedup 1.0000x reference)
"""Trainium2 Bass kernel for nn_RNN_72688026517902.

Math (reference): leaky softplus RNN over T=512 steps,
  u = x @ W_in.T + b_sens + noise*SIGMA
  h_t = (1-ALPHA) h_{t-1} + ALPHA * softplus(u_t + h_{t-1} @ (W_rec*mask).T + b_rec)
  out = h @ W_out.T + b_out

Structure exploited (verified on the actual inputs at runtime, numpy
fallback otherwise): W_rec = I, so Wm = W_rec*mask is DIAGONAL with 0/1
entries d. softplus(z) is computed as Ln(Exp(z) + 1) (both live in one
activation table; this toolchain has no working Softplus table).

  - Channels with d=0 (171 "lin") have a LINEAR recurrence. Since every
    channel shares the same EMA decay, projection and EMA commute:
        out_lin = EMA_t(softplus(P)) @ WO = EMA_t(softplus(P) @ WO)
    so each t-slab is softplus'ed and immediately projected to 33 dims
    (PE), accumulated into y[33, T*BS]; only 33-row EMA scans remain at
    the end. The same applies to the nonlinear channels' a_t history.
  - Channels with d=1 (nnl=85 "nl") run the true sequential chain in a
    scaled form G_t = 5*(h_{t-1}+P_t):
        E   = Exp(0.2*G_t)              (ACT)
        a_t = Ln(E + 1)                 (ACT, +1 via bias)
        G_{t+1} = 0.8*G_t + a_t + Q'_t  (DVE: stt off-path + tt on-path)
    with Q'_t = 5*P_{t+1} - 4*P_t. The input projection produces
    P' = -4*P directly (weights pre-scaled), so Q' = -1.25*P'_next + P'
    is one in-place pass.

Sharding: data-parallel over batch B=256 -> 32 per core on 8 cores.
All layouts are t-major (col = t*32 + b). buf1 holds P'/Q' at 32-col
block t+1 and a_t at block t. Inputs stream in 8 t-slabs of 64 steps;
bulk lin work is dripped into the chain's engine idle gaps.
"""
import sys

sys.path.insert(0, "/opt/trn_rl_repo")
import numpy as np
import ml_dtypes

BF16 = ml_dtypes.bfloat16
T, B = 512, 256
RULE_START, N_RULE, N_OUT, H = 85, 20, 33, 256
IN = RULE_START + N_RULE  # 105
ALPHA, SIGMA = 0.2, 0.05
N_CORES = 8
BS = B // N_CORES        # 32
NSLAB = 8
SLAB = T // NSLAB        # 64
SLABCOL = BS * SLAB      # 2048
NB = T * BS              # 16384

_cache = {}


def _split_channels(d):
    nl = np.where(d != 0.0)[0]
    lin = np.where(d == 0.0)[0]
    nnl = len(nl)
    assert 0 < nnl <= 128 and len(lin) > 128
    return nnl, nl, lin[:128], lin[128:]


def _softplus(z):
    return np.logaddexp(0.0, z)


def np_sim(x, noise, W_sens, b_sens, W_rule, W_rec, b_rec, mask, W_out, b_out):
    """Numpy model of the device dataflow (bf16 noise+staging for lin)."""
    d = np.diag(np.asarray(W_rec) * np.asarray(mask))
    nnl, nl, linA, linB = _split_channels(d)
    W_in = np.concatenate([W_sens, W_rule], axis=1)
    c = np.asarray(b_sens) + np.asarray(b_rec)
    nz = np.asarray(noise).transpose(2, 0, 1).astype(np.float32)
    P = (np.einsum("tbi,hi->htb", x, W_in) + c[:, None, None]).astype(np.float32)
    Pnl = P[nl] + nz[nl] * SIGMA
    G = (5.0 * Pnl[:, 0]).astype(np.float32)
    a_hist = np.zeros_like(Pnl)
    for t in range(T):
        a = _softplus(0.2 * G).astype(np.float32)
        a_hist[:, t] = a
        if t < T - 1:
            G = (0.8 * G + a + 5.0 * Pnl[:, t + 1] - 4.0 * Pnl[:, t]).astype(np.float32)
    WOf = np.asarray(W_out, np.float32)
    y = np.einsum("htb,ho->otb", a_hist, 0.2 * WOf[:, nl].T)
    for ch in (linA, linB):
        nzb = nz[ch].astype(BF16).astype(np.float32)
        Pst = (P[ch] + nzb * SIGMA).astype(BF16)
        E = np.exp(Pst.astype(np.float32)).astype(BF16)
        sp = np.log1p(E.astype(np.float32)).astype(BF16).astype(np.float32)
        y += np.einsum("htb,ho->otb", sp, 0.2 * WOf[:, ch].T)
    out = np.zeros((T, B, N_OUT), np.float32)
    acc = np.zeros((N_OUT, B), np.float32)
    for t in range(T):
        acc = 0.8 * acc + y[:, t]
        out[t] = acc.T
    return (out + np.asarray(b_out)).astype(np.float32)


def _build_program(nnl, nlinB):
    import trn_common
    hook = trn_common.setup()
    from concourse import bacc, mybir
    from concourse.tile import TileContext
    from contextlib import ExitStack

    AF = mybir.ActivationFunctionType
    ALU = mybir.AluOpType
    FP32 = mybir.dt.float32
    BF = mybir.dt.bfloat16

    nc = bacc.Bacc(None)
    xT = nc.declare_dram_parameter("xT", [IN + 1, NB], FP32, isOutput=False)
    nz1 = nc.declare_dram_parameter("nz1", [nnl, NB], FP32, isOutput=False)
    nz2 = nc.declare_dram_parameter("nz2", [128, NB], BF, isOutput=False)
    nz3 = nc.declare_dram_parameter("nz3", [nlinB, NB], BF, isOutput=False)
    W1 = nc.declare_dram_parameter("W1", [IN + 1, nnl], FP32, isOutput=False)
    W2 = nc.declare_dram_parameter("W2", [IN + 1, 128], FP32, isOutput=False)
    W3 = nc.declare_dram_parameter("W3", [IN + 1, nlinB], FP32, isOutput=False)
    WO1 = nc.declare_dram_parameter("WO1", [nnl, N_OUT], FP32, isOutput=False)
    WO2 = nc.declare_dram_parameter("WO2", [128, N_OUT], BF, isOutput=False)
    WO3 = nc.declare_dram_parameter("WO3", [nlinB, N_OUT], BF, isOutput=False)
    bout = nc.declare_dram_parameter("bout", [N_OUT, 1], FP32, isOutput=False)
    bout02 = nc.declare_dram_parameter("bout02", [N_OUT, 1], FP32, isOutput=False)
    OUT = nc.declare_dram_parameter("OUT", [N_OUT, NB], FP32, isOutput=True)

    with TileContext(nc) as tc, ExitStack() as ctx:
        const = ctx.enter_context(tc.tile_pool(name="const", bufs=1))
        big = ctx.enter_context(tc.tile_pool(name="big", bufs=1))
        xstg = ctx.enter_context(tc.tile_pool(name="xstg", bufs=3))
        lstg = ctx.enter_context(tc.tile_pool(name="lstg", bufs=3))
        
        psin = ctx.enter_context(tc.tile_pool(name="psin", bufs=4, space="PSUM"))
        psy = ctx.enter_context(tc.tile_pool(name="psy", bufs=4, space="PSUM"))
        sm = ctx.enter_context(tc.tile_pool(name="sm", bufs=4))

        W1sb = const.tile([IN + 1, nnl], FP32)
        W2sb = const.tile([IN + 1, 128], FP32)
        W3sb = const.tile([IN + 1, nlinB], FP32)
        WO1sb = const.tile([nnl, N_OUT], FP32)
        WO2sb = const.tile([128, N_OUT], BF)
        WO3sb = const.tile([nlinB, N_OUT], BF)
        boutsb = const.tile([N_OUT, 1], FP32)
        bout02sb = const.tile([N_OUT, 1], FP32)
        c08 = const.tile([N_OUT, T], FP32)
        nc.sync.dma_start(W1sb[:], W1[:])
        for (dst, dsrc) in ((W2sb, W2), (W3sb, W3), (WO1sb, WO1),
                            (WO2sb, WO2), (WO3sb, WO3), (boutsb, bout),
                            (bout02sb, bout02)):
            nc.sync.dma_start(dst[:], dsrc[:])
        nc.vector.memset(c08[:], 0.8)

        buf1 = big.tile([nnl, 32 * (T + 1)], FP32)
        F1 = buf1[:]
        ybuf = big.tile([N_OUT, NB], FP32)
        Y = ybuf[:]

        def pblk(t, cnt=1):
            # P'_t .. P'_{t+cnt-1}
            return F1[:, 32 * (t + 1):32 * (t + 1 + cnt)]

        def ablk(t, cnt=1):
            return F1[:, 32 * t:32 * (t + cnt)]

        sched = []          # (gate_step, work item)
        G0_holder = []
        slab_tiles = {}

        def emit_slab_dma(s):
            c0 = s * SLABCOL
            xt = xstg.tile([IN + 1, SLABCOL], FP32, tag="x")
            nc.sync.dma_start(xt[:], xT[:, c0:c0 + SLABCOL])
            nc.sync.dma_start(pblk(SLAB * s, SLAB), nz1[:, c0:c0 + SLABCOL])
            sp2 = lstg.tile([128, SLABCOL], BF, tag="sp2")
            nc.sync.dma_start(sp2[:], nz2[:, c0:c0 + SLABCOL])
            sp3 = lstg.tile([nlinB, SLABCOL], BF, tag="sp3")
            nc.sync.dma_start(sp3[:], nz3[:, c0:c0 + SLABCOL])
            slab_tiles[s] = (xt, sp2, sp3)

        def emit_slab_compute_piece(s, q, j):
            xt, sp2, sp3 = slab_tiles[s]
            sl = slice(q * 512, (q + 1) * 512)
            if j == 0:
                ps = psin.tile([128, 512], FP32, tag="ps")
                nc.tensor.matmul(ps[0:nnl, :], W1sb[:], xt[:, sl], start=True, stop=True)
                dstp = pblk(SLAB * s + 16 * q, 16)
                nc.vector.scalar_tensor_tensor(
                    dstp, dstp, -4.0 * SIGMA, ps[0:nnl, :], ALU.mult, ALU.add)
            else:
                (Wsb, spst, m) = ((W2sb, sp2, 128), (W3sb, sp3, nlinB))[j - 1]
                ps = psin.tile([128, 512], FP32, tag="ps")
                nc.tensor.matmul(ps[0:m, :], Wsb[:], xt[:, sl], start=True, stop=True)
                nc.vector.scalar_tensor_tensor(
                    spst[:, sl], spst[:, sl], SIGMA, ps[0:m, :],
                    ALU.mult, ALU.add)

        def emit_slab_compute_tail(s):
            xt, sp2, sp3 = slab_tiles[s]
            if s == 0:
                G = sm.tile([nnl, BS], FP32, tag="G")
                nc.vector.tensor_scalar(G[:], pblk(0), -1.25, None, ALU.mult)
                G0_holder.append(G)
            if s > 0:
                pr = SLAB * (s - 1)
                nc.vector.scalar_tensor_tensor(
                    pblk(pr, SLAB), pblk(pr + 1, SLAB), -1.25, pblk(pr, SLAB),
                    ALU.mult, ALU.add)
            if s == NSLAB - 1:
                pr = SLAB * s
                nc.vector.scalar_tensor_tensor(
                    pblk(pr, SLAB - 1), pblk(pr + 1, SLAB - 1), -1.25,
                    pblk(pr, SLAB - 1), ALU.mult, ALU.add)
            gate = 0 if s < 2 else SLAB * (s - 2) + 24
            for (spst, m) in ((sp2, 128), (sp3, nlinB)):
                for p in range(0, SLABCOL, 256):
                    sched.append((gate, ("exp", spst, m, p)))
                for p in range(0, SLABCOL, 256):
                    sched.append((gate, ("ln", spst, m, p)))
            for q in range(4):
                sched.append((gate, ("proj", sp2, sp3, s, q)))

        def do_work(w):
            if w[0] in ("exp", "ln"):
                _, spst, m, p = w
                v = spst[0:m, p:p + 256]
                if w[0] == "exp":
                    nc.scalar.activation(v, v, AF.Exp)
                else:
                    nc.scalar.activation(v, v, AF.Ln, bias=1.0)
            else:
                _, sp2, sp3, s, q = w
                sl = slice(s * SLABCOL + q * 512, s * SLABCOL + (q + 1) * 512)
                py = psy.tile([N_OUT, 512], FP32, tag="py")
                nc.tensor.matmul(py[:], WO2sb[:], sp2[:, q * 512:(q + 1) * 512],
                                 start=True, stop=False)
                nc.tensor.matmul(py[:], WO3sb[0:nlinB, :],
                                 sp3[0:nlinB, q * 512:(q + 1) * 512],
                                 start=False, stop=True)
                # first write of this y range: copy + 0.2*b_out (the EMA then
                # carries the bias exactly, with initial=b_out)
                nc.vector.tensor_scalar(Y[:, sl], py[:], 1.0, bout02sb[:, 0:1],
                                        ALU.mult, ALU.add)

        def emit_nl_proj_q(s, q):
            sl = slice(s * SLABCOL + q * 512, s * SLABCOL + (q + 1) * 512)
            py = psy.tile([N_OUT, 512], FP32, tag="py")
            nc.tensor.matmul(py[:], WO1sb[:],
                             ablk(SLAB * s + 16 * q, 16), start=True, stop=True)
            nc.vector.scalar_tensor_tensor(
                Y[:, sl], py[:], 1.0, Y[:, sl], ALU.mult, ALU.add)

        YR = Y.rearrange("p (t b) -> p b t", b=BS)
        def emit_slab_compute(s):
            for q in range(4):
                for j in range(3):
                    emit_slab_compute_piece(s, q, j)
            emit_slab_compute_tail(s)

        for s in range(3):
            emit_slab_dma(s)
        emit_slab_compute(0)
        emit_slab_compute(1)
        G = G0_holder[0]

        wi = 0
        for t in range(T):
            E = sm.tile([nnl, BS], FP32, tag="E")
            nc.scalar.activation(E[:], G[:], AF.Exp, scale=0.2)
            nc.scalar.activation(ablk(t), E[:], AF.Ln, bias=1.0)
            s, tau = t // SLAB, t % SLAB
            if 16 <= tau < 28 and s + 2 < NSLAB:
                k = tau - 16
                emit_slab_compute_piece(s + 2, k // 3, k % 3)
            if tau == 28 and s + 2 < NSLAB:
                emit_slab_compute_tail(s + 2)
            if tau == 32 and s + 3 < NSLAB:
                emit_slab_dma(s + 3)
            if s >= 1 and tau in (24, 28, 32, 36):
                emit_nl_proj_q(s - 1, (tau - 24) // 4)
            if wi < len(sched) and sched[wi][0] <= t:
                do_work(sched[wi][1])
                wi += 1
            if 400 <= t < 400 + BS:
                b = t - 400
                nc.vector.tensor_tensor_scan(
                    YR[:, b, 0:256], c08[:, 0:256], YR[:, b, 0:256],
                    boutsb[:, 0:1], ALU.mult, ALU.add)
            if t < T - 1:
                w = sm.tile([nnl, BS], FP32, tag="w")
                nc.vector.scalar_tensor_tensor(
                    w[:], G[:], 0.8, pblk(t), ALU.mult, ALU.add)
                G2 = sm.tile([nnl, BS], FP32, tag="G")
                nc.vector.tensor_tensor(G2[:], ablk(t), w[:], ALU.add)
                G = G2
        while wi < len(sched):
            do_work(sched[wi][1])
            wi += 1
        for q in range(4):
            emit_nl_proj_q(NSLAB - 1, q)

        # --- final: second-half EMA scans (carry from col t=255) + store
        for b in range(BS):
            nc.vector.tensor_tensor_scan(
                YR[:, b, 256:512], c08[:, 256:512], YR[:, b, 256:512],
                Y[:, 255 * BS + b:255 * BS + b + 1], ALU.mult, ALU.add)
        nc.sync.dma_start(OUT[:], Y[:, :])

    nc.finalize()
    return nc, hook


def kernel(x, noise, W_sens, b_sens, W_rule, W_rec, b_rec, mask, W_out, b_out,
           _profile_dir=None):
    x = np.asarray(x, dtype=np.float32)
    noise = np.asarray(noise, dtype=np.float32)
    Wm = np.asarray(W_rec, dtype=np.float64) * np.asarray(mask, dtype=np.float64)
    d = np.diag(Wm).copy()
    diag_ok = (np.count_nonzero(Wm - np.diag(d)) == 0) and \
        np.all((d == 0.0) | (d == 1.0))
    if not diag_ok:
        W_in = np.concatenate([W_sens, W_rule], axis=1)
        u = np.einsum("tbi,hi->tbh", x, W_in) + np.asarray(b_sens) + noise * SIGMA
        h = np.zeros((B, H), np.float32)
        hs = np.zeros((T, B, H), np.float32)
        Wmf = Wm.astype(np.float32)
        for t in range(T):
            z = u[t] + h @ Wmf.T + np.asarray(b_rec)
            h = (ALPHA * _softplus(z) + (1 - ALPHA) * h).astype(np.float32)
            hs[t] = h
        return (np.einsum("tbh,oh->tbo", hs, W_out) + np.asarray(b_out)).astype(np.float32)

    nnl, nl, linA, linB = _split_channels(d)
    key = ("prog", nnl, len(linB))
    if key not in _cache:
        _cache[key] = _build_program(nnl, len(linB))
    nc, hook = _cache[key]
    from concourse.bass_utils import run_bass_kernel_spmd

    W_in = np.concatenate([W_sens, W_rule], axis=1).astype(np.float32)
    c = (np.asarray(b_sens) + np.asarray(b_rec)).astype(np.float32)

    def wmat(ch, scale=1.0):
        return np.ascontiguousarray(
            scale * np.concatenate([W_in[ch].T, c[ch][None, :]], axis=0), np.float32)

    WOf = np.asarray(W_out, np.float32)
    bouth = np.asarray(b_out, np.float32).reshape(N_OUT, 1)
    in_maps = []
    for cix in range(N_CORES):
        bsl = slice(cix * BS, (cix + 1) * BS)
        xs = x[:, bsl, :]
        xt = np.concatenate(
            [xs.transpose(2, 0, 1).reshape(IN, NB),
             np.ones((1, NB), np.float32)], axis=0)
        nzc = np.ascontiguousarray(noise[:, bsl, :].transpose(2, 0, 1)).reshape(H, NB)
        in_maps.append({
            "xT": np.ascontiguousarray(xt),
            "nz1": np.ascontiguousarray(nzc[nl]),
            "nz2": np.ascontiguousarray(nzc[linA]).astype(BF16),
            "nz3": np.ascontiguousarray(nzc[linB]).astype(BF16),
            "W1": wmat(nl, -4.0), "W2": wmat(linA), "W3": wmat(linB),
            "WO1": np.ascontiguousarray(0.2 * WOf[:, nl].T, np.float32),
            "WO2": np.ascontiguousarray(0.2 * WOf[:, linA].T, np.float32).astype(BF16),
            "WO3": np.ascontiguousarray(0.2 * WOf[:, linB].T, np.float32).astype(BF16),
            "bout": bouth, "bout02": 0.2 * bouth,
        })

    if _profile_dir is not None and hook is not None:
        with hook(_profile_dir, list(range(N_CORES))):
            res = run_bass_kernel_spmd(nc, in_maps, list(range(N_CORES)))
    else:
        res = run_bass_kernel_spmd(nc, in_maps, list(range(N_CORES)))

    out = np.empty((T, B, N_OUT), np.float32)
    for cix in range(N_CORES):
        oc = res.results[cix]["OUT"]
        out[:, cix * BS:(cix + 1) * BS, :] = oc.reshape(N_OUT, T, BS).transpose(1, 2, 0)
    return out
